# revision 29
# baseline (speedup 1.0000x reference)
"""Trainium2 Bass kernel for nn_EntityBase (sparse entity attention MLP).

Math (per bs*ts element, 2048 total):
  x1   = relu(x @ W1.T + b1)                       x:[64,128] -> x1:[64,512]
  qkv  = x1 @ Win.T ; q = qkv[:, :512][:16 agents], k, v ; heads 8 x 64
  lg   = (q . k)/8 masked with obs_mask (NEG), softmax over keys,
         fully-masked rows -> 0
  attn = (w @ v) @ Wout.T + b_out, agent-masked to 0
  out  = relu(relu(attn) @ W2.T + b2)              -> [16, 512]

Distribution: data-parallel over the 2048 flattened bs*ts elements across
8 NeuronCores (256 elements/core); weights replicated.

Device dataflow (per core, 16 blocks of 16 elements):
  - entities transposed on HOST to feature-major [128, 16384]; all big GEMMs
    fp32r with N>=256 (full-rate PE)
  - attention in fp16: kT/vT/qT converted to fp16 during their PSUM->SBUF
    copies; logits per (pair, head) via K=64 PE-tile matmuls (head-slices at
    partition base 0/64), output [128 keys-of-pair, (h,q)] in PSUM
  - softmax per half-group (2 pairs, [128,512]): DVE mask-bias add, ACT exp,
    PE ones-matmul partition sum, DVE eps+reciprocal, PE broadcast matmul,
    DVE normalize to fp16 weights
  - attnV per (pair, head-half): K=128 matmuls writing disjoint PSUM
    partition ranges via tile_position (no extraction copies)
  - Wout feature-major + agent mask (broadcast on device); W2 emits
    token-major output directly
"""
import sys
for _p in ("/opt/trn_rl_repo", "/root/.axon_site/_ro/trn_rl_repo"):
    if _p not in sys.path:
        sys.path.insert(0, _p)

import numpy as np
import concourse.bass as bass
import concourse.tile as tile
from concourse import bass_isa, mybir, bacc
from concourse.bass_utils import run_bass_kernel_spmd

FP32 = mybir.dt.float32
FP32R = mybir.dt.float32r
FP16 = mybir.dt.float16
AF = mybir.ActivationFunctionType
ADD = mybir.AluOpType.add
MULT = mybir.AluOpType.mult

# problem dims (hardcoded per spec)
B, T, NE, ED = 32, 64, 64, 128
NA, E, H, R = 16, 512, 8, 512
HD = E // H
NEG = np.float32(-1e30)
NCORES = 8
BT = B * T                     # 2048
NB = BT // NCORES              # 256 elements per core
NTOK = NB * NE                 # 16384 tokens per core
NAG = NB * NA                  # 4096 agent tokens per core
NBLK = 16                      # blocks per core (16 elements each)


def _build_nc(nrep=1):
    nc = bacc.Bacc("TRN2", target_bir_lowering=False, debug=False)
    ap = lambda n, s, d, k: nc.dram_tensor(n, s, d, kind=k).ap()
    entT = ap("entT", [ED, NTOK], FP32R, "ExternalInput")   # host-transposed
    w1t = ap("w1t", [ED, E], FP32R, "ExternalInput")        # W1.T
    b1c = ap("b1c", [128, 4], FP32, "ExternalInput")        # b1 chunked
    wqe = ap("wqe", [E, E], FP32R, "ExternalInput")  # (Win_q/8).T odd cols 0
    wqo = ap("wqo", [E, E], FP32R, "ExternalInput")  # (Win_q/8).T even cols 0
    wkt = ap("wkt", [E, E], FP32R, "ExternalInput")         # Win_k.T
    wvt = ap("wvt", [E, E], FP32R, "ExternalInput")         # Win_v.T
    wot = ap("wot", [E, E], FP32R, "ExternalInput")         # Wout.T
    boc = ap("boc", [128, 4], FP32, "ExternalInput")        # b_out chunked
    w2t = ap("w2t", [E, R], FP32R, "ExternalInput")         # W2.T
    b2r = ap("b2r", [1, R], FP32R, "ExternalInput")
    batt = ap("batt", [NBLK * 128, 256], FP32, "ExternalInput")  # mask bias
    ntg = ap("ntg", [1, NBLK * 256], FP32R, "ExternalInput")  # 1-agent_mask
    onc = ap("onc", [128, 1], FP32R, "ExternalInput")
    onr = ap("onr", [1, 128], FP32R, "ExternalInput")
    out = ap("out", [NAG, R], FP32, "ExternalOutput")

    from contextlib import ExitStack
    POOLS = dict(wp=1, entp=3, x1p=1, ktp=2, vtp=2, qp=2, exp_=4, wnp=2,
                 smallp=2, attp=2, srp=1, otp=2, biasp=2, ntgsp=2, ntgq=2)
    PSUM_POOLS = dict(ps_big=4, ps_lg=2, ps_at=2)
    with tile.TileContext(nc) as tc:
        with ExitStack() as ctx:
            ctx.enter_context(nc.allow_low_precision(
                reason="fp32r/fp16 matmul pipeline by design"))
            pools = {}
            for nm, bufs in POOLS.items():
                pools[nm] = ctx.enter_context(tc.tile_pool(name=nm, bufs=bufs))
            for nm, bufs in PSUM_POOLS.items():
                pools[nm] = ctx.enter_context(
                    tc.tile_pool(name=nm, bufs=bufs, space="PSUM"))
            aps = dict(entT=entT, w1t=w1t, b1c=b1c, wqe=wqe, wqo=wqo,
                       wkt=wkt, wvt=wvt, wot=wot, boc=boc, w2t=w2t, b2r=b2r,
                       batt=batt, ntg=ntg, onc=onc, onr=onr, out=out)
            if nrep == 1:
                _emit(nc, pools, aps)
            else:
                with tc.For_i(0, nrep) as _i:
                    _emit(nc, pools, aps)
    nc.compile()
    return nc


def _emit(nc, pools, aps):
    wp, entp, x1p, ktp, vtp, qp = (pools[k] for k in
                                   ("wp", "entp", "x1p", "ktp", "vtp", "qp"))
    exp_, wnp, smallp, attp, srp, otp, biasp = (
        pools[k] for k in ("exp_", "wnp", "smallp", "attp", "srp", "otp",
                           "biasp"))
    ps_big, ps_lg, ps_at, ntgsp = (
        pools[k] for k in ("ps_big", "ps_lg", "ps_at", "ntgsp"))
    ntgq = pools["ntgq"]
    (entT, w1t, b1c, wqe_d, wqo_d, wkt, wvt, wot, boc, w2t, b2r, batt, ntg,
     onc, onr, out) = (
        aps[k] for k in ("entT", "w1t", "b1c", "wqe", "wqo", "wkt", "wvt",
                         "wot", "boc", "w2t", "b2r", "batt", "ntg", "onc",
                         "onr", "out"))
    if True:
        if True:
            st = {}   # per-block state for the software pipeline

            def preload(blk):
                ent_t = entp.tile([128, 1024], FP32R, tag="ent", name="ent")
                for hh in range(2):
                    nc.sync.dma_start(
                        ent_t[:, hh * 512:(hh + 1) * 512],
                        entT[:, blk * 1024 + hh * 512:
                             blk * 1024 + (hh + 1) * 512])
                bia = biasp.tile([128, 256], FP32, tag="bia", name="bia")
                nc.sync.dma_start(bia[:], batt[blk * 128:(blk + 1) * 128, :])
                ntr = ntgq.tile([1, 256], FP32R, tag="ntr", name="ntr")
                nc.sync.dma_start(ntr[:], ntg[:, blk * 256:(blk + 1) * 256])
                st[blk] = dict(ent_t=ent_t, bia=bia, ntr=ntr)

            # ---- resident weights/constants, ordered by first use; the
            # first block's inputs are queued before the bulk weights ----
            w1s = wp.tile([128, E], FP32R, tag="w1s", name="w1s")
            nc.sync.dma_start(w1s[:], w1t)
            b1s = wp.tile([128, 4], FP32, tag="b1s", name="b1s")
            nc.sync.dma_start(b1s[:], b1c)
            preload(0)
            wqel, wqol, wk, wv, wo, w2 = [], [], [], [], [], []
            wlists = dict(wqe=wqel, wqo=wqol, wk=wk, wv=wv, wo=wo, w2=w2)
            for nm, src in (("wk", wkt), ("wv", wvt), ("wqe", wqe_d),
                            ("wqo", wqo_d)):
                for e in range(4):
                    t_ = wp.tile([128, 512], FP32R, tag=f"{nm}{e}",
                                 name=f"{nm}{e}")
                    nc.sync.dma_start(t_[:], src[e * 128:(e + 1) * 128, :])
                    wlists[nm].append(t_)
            oc = wp.tile([128, 1], FP32R, tag="oc", name="oc")
            nc.sync.dma_start(oc[:], onc)
            orw = wp.tile([1, 128], FP32R, tag="orw", name="orw")
            nc.sync.dma_start(orw[:], onr)
            for nm, src in (("wo", wot), ("w2", w2t)):
                for e in range(4):
                    t_ = wp.tile([128, 512], FP32R, tag=f"{nm}{e}",
                                 name=f"{nm}{e}")
                    nc.sync.dma_start(t_[:], src[e * 128:(e + 1) * 128, :])
                    wlists[nm].append(t_)
            bos = wp.tile([128, 4], FP32, tag="bos", name="bos")
            nc.sync.dma_start(bos[:], boc)
            b2s = wp.tile([1, R], FP32R, tag="b2s", name="b2s")
            nc.sync.dma_start(b2s[:], b2r)

            def phase_A(blk):
                # fc1 + k + v + q GEMMs (inputs DMA'd by preload)
                if blk not in st:
                    preload(blk)
                ent_t, bia = st[blk]["ent_t"], st[blk]["bia"]

                x1T = [x1p.tile([128, 1024], FP32R, tag=f"x1T{m}", name=f"x1T{m}")
                       for m in range(4)]
                for h in range(2):
                    for m in range(4):
                        p = ps_big.tile([128, 512], FP32, tag="big", name="big")
                        nc.tensor.matmul(
                            p[:], w1s[:, m * 128:(m + 1) * 128],
                            ent_t[:, h * 512:(h + 1) * 512])
                        nc.scalar.activation(
                            x1T[m][:, h * 512:(h + 1) * 512], p[:],
                            AF.Relu, bias=b1s[:, m:m + 1])

                kT = [[None] * 4 for _ in range(2)]
                vt = [[None] * 4 for _ in range(2)]
                for g in range(2):
                    for m in range(4):
                        p = ps_big.tile([128, 512], FP32, tag="big", name="big")
                        for e in range(4):
                            nc.tensor.matmul(
                                p[:], wk[e][:, m * 128:(m + 1) * 128],
                                x1T[e][:, g * 512:(g + 1) * 512],
                                start=(e == 0), stop=(e == 3))
                        t_ = ktp.tile([128, 512], FP32R, tag=f"kT{g}{m}",
                                      name=f"kT{g}{m}")
                        nc.vector.tensor_copy(t_[:], p[:])
                        kT[g][m] = t_
                    for pr in range(4):
                        p = ps_big.tile([128, 512], FP32, tag="big", name="big")
                        r0 = g * 512 + pr * 128
                        for e in range(4):
                            nc.tensor.matmul(
                                p[:], x1T[e][:, r0:r0 + 128], wv[e][:],
                                start=(e == 0), stop=(e == 3))
                        t_ = vtp.tile([128, 512], FP32R, tag=f"vt{g}{pr}",
                                      name=f"vt{g}{pr}")
                        nc.scalar.activation(t_[:], p[:], AF.Copy)
                        vt[g][pr] = t_

                # q: two zero-masked variants, interleaved per pair:
                # q_eo[m] cols = (pair 8)(eo 2)(e' 2)(q 16)
                qsb = []
                for m in range(4):
                    pf = ps_big.tile([128, 512], FP32, tag="big", name="big")
                    for vi, wsel in ((0, wqel), (1, wqol)):
                        p = pf[:, vi * 256:(vi + 1) * 256]
                        for e in range(4):
                            agents = x1T[e][:].rearrange(
                                "p (el t) -> p el t", el=16)[:, :, 0:NA]
                            nc.tensor.matmul(
                                p, wsel[e][:, m * 128:(m + 1) * 128], agents,
                                start=(e == 0), stop=(e == 3))
                    t_ = qp.tile([128, 512], FP32R, tag=f"q{m}", name=f"q{m}")
                    dst = t_[:].rearrange("a (p eo e q) -> a p eo e q",
                                          p=8, eo=2, e=2)
                    srcv = pf[:].rearrange("a (eo p e q) -> a eo p e q",
                                           eo=2, p=8, e=2)
                    nc.scalar.activation(dst[:, :, 0], srcv[:, 0], AF.Copy)
                    nc.scalar.activation(dst[:, :, 1], srcv[:, 1], AF.Copy)
                    qsb.append(t_)
                st[blk].update(kT=kT, vt=vt, qsb=qsb)

            def phase_L(blk):
                # logits + mask bias + exp, 4 half-groups
                import os
                sub = os.environ.get("KSUB", "")
                s = st[blk]
                bia, kT, qsb = s["bia"], s["kT"], s["qsb"]
                lgs, exs = [], []
                for hgi in range(4):
                    g, ph = hgi // 2, hgi % 2
                    lg = ps_lg.tile([128, 512], FP32, tag="lg", name="lg")
                    for pr2 in range(2):
                        p4 = ph * 2 + pr2    # pair in group
                        pg = g * 4 + p4      # pair in block (0..7)
                        for m in range(4):
                            nc.tensor.matmul(
                                lg[:, pr2 * 256 + m * 64:
                                   pr2 * 256 + (m + 1) * 64],
                                kT[g][m][:, p4 * 128:
                                         (p4 + 1) * 128].bitcast(FP32),
                                qsb[m][:, pg * 64:
                                       (pg + 1) * 64].bitcast(FP32))
                    if sub == "L1":
                        continue
                    nc.vector.tensor_tensor(
                        lg[:].rearrange("p (pr h q) -> p pr h q", pr=2, h=8),
                        lg[:].rearrange("p (pr h q) -> p pr h q", pr=2, h=8),
                        bia[:, hgi * 64:(hgi + 1) * 64].rearrange(
                            "p (pr q) -> p pr q", pr=2
                        ).unsqueeze(2).broadcast_to([128, 2, 8, 32]),
                        ADD)
                    if sub == "L2":
                        continue
                    ex = exp_.tile([128, 512], FP32R, tag="ex", name="ex")
                    nc.scalar.activation(ex[:], lg[:], AF.Exp)
                    lgs.append(lg)
                    exs.append(ex)
                s["exs"] = exs

            def phase_SCV(blk):
                # softmax sums (gpsimd all-reduce), reciprocals, attnV
                s = st[blk]
                exs, vt = s["exs"], s["vt"]
                wns = []
                for hgi in range(4):
                    dn = smallp.tile([128, 512], FP32, tag="dn", name="dn")
                    nc.gpsimd.partition_all_reduce(
                        dn[:], exs[hgi][:], channels=128,
                        reduce_op=bass_isa.ReduceOp.add)
                    nc.gpsimd.tensor_scalar_add(dn[:], dn[:], 1e-30)
                    r2 = smallp.tile([128, 512], FP32, tag="r2", name="r2")
                    nc.vector.reciprocal(r2[:], dn[:])
                    wn = wnp.tile([128, 512], FP32, tag="wn", name="wn")
                    nc.vector.tensor_tensor(wn[:], exs[hgi][:].bitcast(FP32),
                                            r2[:], MULT)
                    wns.append(wn)
                ats = []
                attnT = [attp.tile([128, 256], FP32R, tag=f"attnT{m}",
                                   name=f"attnT{m}") for m in range(4)]
                for g in range(2):
                    at = ps_at.tile([128, 512], FP32, tag="at", name="at")
                    ats.append(at)
                    for ph in range(2):
                        wn = wns[g * 2 + ph]
                        for pr2 in range(2):
                            p4 = ph * 2 + pr2
                            for m in range(4):
                                nc.tensor.matmul(
                                    at[0:64, m * 128 + p4 * 32:
                                       m * 128 + p4 * 32 + 32],
                                    vt[g][p4][:, m * 128:
                                              m * 128 + 64].bitcast(FP32),
                                    wn[:, pr2 * 256 + (2 * m) * 32:
                                       pr2 * 256 + (2 * m) * 32 + 32])
                                nc.tensor.matmul(
                                    at[64:128, m * 128 + p4 * 32:
                                       m * 128 + p4 * 32 + 32],
                                    vt[g][p4][:, m * 128 + 64:
                                              (m + 1) * 128].bitcast(FP32),
                                    wn[:, pr2 * 256 + (2 * m + 1) * 32:
                                       pr2 * 256 + (2 * m + 1) * 32 + 32])
                    for m in range(4):
                        nc.scalar.activation(
                            attnT[m][:, g * 128:(g + 1) * 128],
                            at[:, m * 128:(m + 1) * 128], AF.Copy)
                # agent-mask broadcast: [1,256] -> [128,256] -> SBUF
                ntgpf = ps_big.tile([128, 512], FP32, tag="big", name="big")
                nc.tensor.matmul(ntgpf[:, 0:256], orw[:], s["ntr"][:])
                ntg_s = ntgsp.tile([128, 256], FP32, tag="ntg_s", name="ntg_s")
                nc.scalar.activation(ntg_s[:], ntgpf[:, 0:256], AF.Copy)
                s["attnT"] = attnT
                s["ntg_s"] = ntg_s

            def phase_O(blk):
                # Wout + mask, W2 + out DMA
                s = st.pop(blk)
                attnT, ntg_s = s["attnT"], s["ntg_s"]
                sr = []
                for m in range(4):
                    pf = ps_big.tile([128, 512], FP32, tag="big", name="big")
                    p = pf[:, 0:256]
                    for e in range(4):
                        nc.tensor.matmul(
                            p, wo[e][:, m * 128:(m + 1) * 128],
                            attnT[e][:], start=(e == 0), stop=(e == 3))
                    t_ = srp.tile([128, 256], FP32R, tag=f"sr{m}", name=f"sr{m}")
                    nc.scalar.activation(t_[:], p, AF.Relu,
                                         bias=bos[:, m:m + 1])
                    nc.vector.tensor_tensor(t_[:], t_[:].bitcast(FP32),
                                            ntg_s[:], MULT)
                    sr.append(t_)
                for t in range(2):
                    p = ps_big.tile([128, 512], FP32, tag="big", name="big")
                    nc.tensor.matmul(p[:], orw[:], b2s[:],
                                     start=True, stop=False,
                                     skip_group_check=True)
                    for e in range(4):
                        nc.tensor.matmul(
                            p[:], sr[e][:, t * 128:(t + 1) * 128], w2[e][:],
                            start=False, stop=(e == 3), skip_group_check=True)
                    ot = otp.tile([128, 512], FP32, tag="ot", name="ot")
                    nc.scalar.activation(ot[:], p[:], AF.Relu)
                    r0 = blk * 256 + t * 128
                    nc.sync.dma_start(out[r0:r0 + 128, :], ot[:])

            # software pipeline: A(b), SCV(b-1), L(b), O(b-1)
            import os
            nblk = int(os.environ.get("KBLKS", NBLK))
            kphase = os.environ.get("KPHASE", "O")
            phase_A(0)
            if kphase == "A":
                return
            if nblk > 1:
                phase_A(1)
            phase_L(0)
            if kphase == "L":
                return
            if kphase == "SCV":
                phase_SCV(0)
                return
            for blk in range(1, nblk):
                if blk + 1 < nblk:
                    phase_A(blk + 1)
                if blk + 2 < nblk:
                    preload(blk + 2)
                phase_SCV(blk - 1)
                phase_O(blk - 1)
                phase_L(blk)
            phase_SCV(nblk - 1)
            phase_O(nblk - 1)


_NC_CACHE = None

def _get_nc(nrep=1):
    global _NC_CACHE
    if _NC_CACHE is None:
        _NC_CACHE = {}
    if nrep not in _NC_CACHE:
        _NC_CACHE[nrep] = _build_nc(nrep)
    return _NC_CACHE[nrep]


def _prep_in_maps(entities, obs_mask, entity_mask, W1, b1, Win, Wout, b_out,
                  W2, b2):
    f32 = np.float32
    ent = np.asarray(entities, f32).reshape(BT, NE, ED)
    pre = np.asarray(obs_mask).reshape(BT, NE, NE)[:, :NA, :]   # [2048,16,64]
    agm = np.asarray(entity_mask).reshape(BT, NE)[:, :NA]       # [2048,16]
    W1, b1 = np.asarray(W1, f32), np.asarray(b1, f32)
    Win, Wout = np.asarray(Win, f32), np.asarray(Wout, f32)
    b_out, W2, b2 = np.asarray(b_out, f32), np.asarray(W2, f32), np.asarray(b2, f32)

    wq_t = (Win[0:E] * np.float32(1.0 / np.sqrt(HD))).T   # [e, f]
    fidx = np.arange(E)
    wq_even = wq_t.copy(); wq_even[:, (fidx // HD) % 2 == 1] = 0.0
    wq_odd = wq_t.copy(); wq_odd[:, (fidx // HD) % 2 == 0] = 0.0
    shared = {
        "w1t": np.ascontiguousarray(W1.T),
        "b1c": np.ascontiguousarray(b1.reshape(4, 128).T),
        "wqe": np.ascontiguousarray(wq_even),
        "wqo": np.ascontiguousarray(wq_odd),
        "wkt": np.ascontiguousarray(Win[E:2 * E].T),
        "wvt": np.ascontiguousarray(Win[2 * E:3 * E].T),
        "wot": np.ascontiguousarray(Wout.T),
        "boc": np.ascontiguousarray(b_out.reshape(4, 128).T),
        "w2t": np.ascontiguousarray(W2.T),
        "b2r": np.ascontiguousarray(b2.reshape(1, R)),
        "onc": np.ones((128, 1), f32),
        "onr": np.ones((1, 128), f32),
    }
    in_maps = []
    for c in range(NCORES):
        s = slice(c * NB, (c + 1) * NB)
        entT_c = np.ascontiguousarray(
            ent[s].reshape(NTOK, ED).T)                   # [128, 16384]
        # attention bias per pair: [128 (2e x 64k), (2e' x 16q)]
        obsT = pre[s].astype(f32).transpose(0, 2, 1)      # [256, 64k, 16q]
        bias = np.full((NB // 2, 2, 64, 2, 16), NEG, f32)
        bias[:, 0, :, 0, :] = NEG * obsT[0::2]
        bias[:, 1, :, 1, :] = NEG * obsT[1::2]
        bias = bias.reshape(NB // 2, 128, 32)             # [128 pairs,128,32]
        # regroup to per-block [16, 128, (8 pairs x 32)]
        bias = np.ascontiguousarray(
            bias.reshape(NBLK, 8, 128, 32).transpose(0, 2, 1, 3)
        ).reshape(NBLK * 128, 256)
        ntg_c = np.ascontiguousarray(
            (1.0 - agm[s].astype(f32)).reshape(1, NBLK * 256))
        m = dict(shared)
        m["entT"] = entT_c
        m["batt"] = bias
        m["ntg"] = ntg_c
        in_maps.append(m)
    return in_maps


def kernel(**inputs) -> np.ndarray:
    nc = _get_nc()
    in_maps = _prep_in_maps(**inputs)
    res = run_bass_kernel_spmd(nc, in_maps, list(range(NCORES)))
    outs = [res.results[c]["out"] for c in range(NCORES)]       # [4096, 512]
    full = np.concatenate(outs, axis=0).reshape(BT, NA, R)
    return np.ascontiguousarray(full.reshape(B, T, NA, R)).astype(np.float32)
